# revision 14
# baseline (speedup 1.0000x reference)
"""Trainium2 Bass kernel for BatchedCrossAttentionXSMM.

Reference computation (B=1, NQ=NK=2048, A=M=1024, H=16, KD=VD=64):
    q = (q_data @ query_w + query_b) * kd^-0.5      [Q, H, KD]
    k = m_data @ key_w                               [K, H, KD]
    v = m_data @ value_w                             [K, H, VD]
    logits = q k^T + bias                            [H, Q, K]
    w = softmax(logits, axis=-1)
    out = sigmoid(q_data @ gating_w) * (w @ v)       [Q, H, VD]

Sharding: tensor-parallel over heads -- 2 heads per NeuronCore, 8 cores.
Each core gets the full activations (pre-transposed + bf16 on host) and its
2 heads' weight/bias slices; outputs are concatenated on the host.

On-device layout trick: logits are computed *transposed* ([k, q]) as
K Q^T, accumulated on top of PSUM pre-seeded with bias^T (host supplies
bias already transposed; seeding is an identity matmul so `has_written`
semantics allow the K Q^T accumulation).  exp() then lands E^T in SBUF
directly in the layout the PV matmul needs; a [v | 1] stationary operand
produces both weighted values and softmax denominators in one PE pass.
"""

import re
import sys

for _p in ("/opt/trn_rl_repo",):
    if _p not in sys.path:
        sys.path.insert(0, _p)

import ml_dtypes
import numpy as np

import concourse.bass as bass
import concourse.mybir as mybir
import concourse.tile as tile
from concourse.bass_utils import run_bass_kernel_spmd
from concourse.masks import make_identity

BF16 = ml_dtypes.bfloat16
dt = mybir.dt

NCORES = 8
H_PER_CORE = 2
NQ = NK = 2048
A_DIM = 1024
KD = VD = 64
HC = H_PER_CORE * KD  # 128
SCALE = float(KD) ** -0.5
P = 128
AT = A_DIM // P  # 8 a-subtiles
QT = NQ // P     # 16 token tiles
KT = NK // P


# --- Tile tail-drain patch -------------------------------------------------
# The walrus build in this image caps sem-waits per instruction at 2; Tile's
# kernel-tail drain attaches one wait per live semaphore to a single Drain,
# which fails codegen ("Too many sync wait commands").  Spread the waits over
# a chain of SP nops (1 wait each) before the drain instead.
def _patched_drain_and_barrier(self, tick_clock, wait_clock):
    nc = self.nc
    gc = tick_clock.global_clock
    vals = [int(v) for v in re.findall(r"\d+", repr(gc))]
    alloc = self.sems.allocated()
    waits = []
    for proc, sem in alloc.items():
        v = vals[proc] if proc < len(vals) else 0
        if v > 0:
            mult = 16 if "DMA" in sem.name else 1
            waits.append((sem, v * mult))
    for sem, val in waits:
        nc.sync.nop(nofuse=True).wait_op(sem, val, "sem-ge")
    nc.sync.drain()
    nc.all_engine_barrier()
    popped = nc._tile_sem_poison_stack.pop()
    assert popped is self._sem_poison
    nc.clear_and_free_semaphores(list(self.sems.allocated().values()))
    nc.all_engine_barrier()


tile.TileContext._drain_and_barrier = _patched_drain_and_barrier


# --- BIR wait-splitting pass ----------------------------------------------
# Tile's wait assignment can attach 3+ semaphore waits to a single
# instruction; this walrus build encodes at most 2 wait commands per
# instruction.  Rewrite the serialized BIR: hoist excess waits onto
# preceding EventSemaphore instructions on the same engine.
_MAXW = 1
_orig_to_json_bytes = bass.Bass.to_json_bytes


def _to_json_bytes_split_waits(self):
    import json

    data = json.loads(_orig_to_json_bytes(self))
    ctr = 0
    for fn in data.get("functions", []):
        for bb in fn.get("blocks", []):
            newl = []
            for ins in bb["instructions"]:
                si = ins.get("sync_info")
                if si and si.get("on_wait") and len(si["on_wait"]) > _MAXW:
                    waits = si["on_wait"]
                    extra, keep = waits[:-_MAXW], waits[-_MAXW:]
                    for i in range(0, len(extra), _MAXW):
                        ctr += 1
                        newl.append({
                            "debug": ins.get("debug", 0),
                            "engine": ins["engine"],
                            "ins": [],
                            "outs": [],
                            "name": f"{ins['name']}-wsplit{ctr}",
                            "opcode": "EventSemaphore",
                            "sync_info": {
                                "on_update": [],
                                "on_wait": extra[i:i + _MAXW],
                            },
                        })
                    si["on_wait"] = keep
                newl.append(ins)
            bb["instructions"] = newl
    return json.dumps(data).encode()


bass.Bass.to_json_bytes = _to_json_bytes_split_waits


# --- device program --------------------------------------------------------
def build_nc():
    nc = bass.Bass()
    f32, bf16 = dt.float32, dt.bfloat16

    qT_d = nc.dram_tensor("qT", [A_DIM, NQ], bf16, kind="ExternalInput")
    mT_d = nc.dram_tensor("mT", [A_DIM, NK], bf16, kind="ExternalInput")
    biasT_d = nc.dram_tensor("biasT", [H_PER_CORE, NK, NQ], bf16, kind="ExternalInput")
    wq_d = nc.dram_tensor("wq", [A_DIM, HC], bf16, kind="ExternalInput")
    wk_d = nc.dram_tensor("wk", [A_DIM, HC], bf16, kind="ExternalInput")
    wv_d = nc.dram_tensor("wv", [A_DIM, HC], bf16, kind="ExternalInput")
    wg_d = nc.dram_tensor("wg", [A_DIM, HC], bf16, kind="ExternalInput")
    bq_d = nc.dram_tensor("bq", [HC, 1], f32, kind="ExternalInput")
    o_d = nc.dram_tensor("o", [NQ, HC], f32, kind="ExternalOutput")

    with tile.TileContext(nc) as tc:
        with (
            tc.tile_pool(name="consts", bufs=1) as consts,
            tc.tile_pool(name="bigp", bufs=3) as bigp,
            tc.tile_pool(name="etp", bufs=4) as etp,
            tc.tile_pool(name="wsbp", bufs=2) as wsbp,
            tc.tile_pool(name="smallp", bufs=4) as smallp,
            tc.tile_pool(name="pL", bufs=3, space="PSUM") as pL,
            tc.tile_pool(name="pW", bufs=2, space="PSUM") as pW,
        ):
            # ---- constants / persistent SBUF ----
            qT_sb = bigp.tile([P, AT, NQ], bf16, tag="big")
            mT_sb = bigp.tile([P, AT, NK], bf16, tag="big")
            nc.sync.dma_start(out=qT_sb, in_=qT_d[:, :].rearrange("(at p) n -> p at n", p=P))
            nc.sync.dma_start(out=mT_sb, in_=mT_d[:, :].rearrange("(at p) n -> p at n", p=P))

            w_sb = {}
            for name, d in (("wq", wq_d), ("wk", wk_d), ("wv", wv_d), ("wg", wg_d)):
                t = consts.tile([P, AT, HC], bf16, tag=f"{name}_sb")
                nc.sync.dma_start(out=t, in_=d[:, :].rearrange("(at p) m -> p at m", p=P))
                w_sb[name] = t
            bq_sb = consts.tile([HC, 1], f32, tag="bq_sb")
            nc.sync.dma_start(out=bq_sb, in_=bq_d[:, :])

            id_bf = consts.tile([P, P], bf16, tag="id_bf")
            make_identity(nc, id_bf)
            id_f32 = consts.tile([P, P], f32, tag="id_f32")
            make_identity(nc, id_f32)

            qT2 = consts.tile([HC, NQ], bf16, tag="qT2")
            kT2 = consts.tile([HC, NK], bf16, tag="kT2")
            v_sb = consts.tile([P, H_PER_CORE, KT, VD + 1], bf16, tag="v_sb")
            gate_sb = consts.tile([P, QT, HC], f32, tag="gate_sb")
            out_sb = consts.tile([P, QT, HC], f32, tag="out_sb")

            vT2 = consts.tile([HC, NK], bf16, tag="vT2")
            gT2 = consts.tile([HC, NQ], bf16, tag="gT2")

            # ---- phase 1: projections ----
            # All four projections in [head*c, token] layout: stationary is the
            # weight slice (8 LDWs each), the pre-transposed activations stream.
            for ch in range(2):
                sl = slice(ch * 1024, (ch + 1) * 1024)
                ps_q = pL.tile([P, 1024], f32, tag="pl")
                ps_k = pL.tile([P, 1024], f32, tag="pl")
                for at in range(AT):
                    for hf in range(2):
                        fs = slice(hf * 512, (hf + 1) * 512)
                        gfs = slice(ch * 1024 + hf * 512, ch * 1024 + (hf + 1) * 512)
                        st, sp = (at == 0), (at == AT - 1)
                        nc.tensor.matmul(ps_q[:, fs], lhsT=w_sb["wq"][:, at, :],
                                         rhs=qT_sb[:, at, gfs], start=st, stop=sp)
                        nc.tensor.matmul(ps_k[:, fs], lhsT=w_sb["wk"][:, at, :],
                                         rhs=mT_sb[:, at, gfs], start=st, stop=sp)
                # (q + bq) * scale, cast to bf16
                nc.vector.tensor_scalar(
                    out=qT2[:, sl], in0=ps_q, scalar1=bq_sb, scalar2=SCALE,
                    op0=mybir.AluOpType.add, op1=mybir.AluOpType.mult,
                )
                nc.vector.tensor_copy(out=kT2[:, sl], in_=ps_k)
            for ch in range(2):
                sl = slice(ch * 1024, (ch + 1) * 1024)
                ps_v = pL.tile([P, 1024], f32, tag="pl")
                ps_g = pL.tile([P, 1024], f32, tag="pl")
                for at in range(AT):
                    for hf in range(2):
                        fs = slice(hf * 512, (hf + 1) * 512)
                        gfs = slice(ch * 1024 + hf * 512, ch * 1024 + (hf + 1) * 512)
                        st, sp = (at == 0), (at == AT - 1)
                        nc.tensor.matmul(ps_v[:, fs], lhsT=w_sb["wv"][:, at, :],
                                         rhs=mT_sb[:, at, gfs], start=st, stop=sp)
                        nc.tensor.matmul(ps_g[:, fs], lhsT=w_sb["wg"][:, at, :],
                                         rhs=qT_sb[:, at, gfs], start=st, stop=sp)
                nc.vector.tensor_copy(out=vT2[:, sl], in_=ps_v)
                # sigmoid(x) = 0.5 + 0.5*tanh(x/2): tanh shares the exp ACT
                # table set, so no table reload between phases.
                nc.scalar.activation(out=gT2[:, sl], in_=ps_g,
                                     func=mybir.ActivationFunctionType.Tanh,
                                     scale=0.5)
                nc.vector.tensor_scalar(
                    out=gT2[:, sl], in0=gT2[:, sl], scalar1=0.5, scalar2=0.5,
                    op0=mybir.AluOpType.mult, op1=mybir.AluOpType.add,
                )

            # transpose v^T back to natural [k, c] tiles with a ones column
            # (both heads in one full-128 transpose per k-tile).
            for kt in range(KT):
                tps = pL.tile([P, P], f32, tag="pl", name=f"vtp{kt}")
                nc.tensor.matmul(
                    tps, lhsT=vT2[:, kt * P:(kt + 1) * P],
                    rhs=id_bf, start=True, stop=True,
                )
                nc.vector.tensor_copy(
                    out=v_sb[:, :, kt, 0:VD],
                    in_=tps.rearrange("p (h c) -> p h c", h=H_PER_CORE),
                )
            nc.vector.memset(v_sb[:, :, :, VD:VD + 1], 1.0)

            # transpose gate^T back to natural [q, head*c] (both heads at once).
            for qt in range(QT):
                tps = pL.tile([P, P], f32, tag="pl")
                nc.tensor.matmul(
                    tps, lhsT=gT2[:, qt * P:(qt + 1) * P], rhs=id_bf,
                    start=True, stop=True,
                )
                nc.vector.tensor_copy(out=gate_sb[:, qt, :], in_=tps)

            # ---- phase 2: attention (q in quarters, heads paired) ----
            # Both heads' logits live in one [P, 2, 512] PSUM tile: the KQ^T
            # pair is row-group packed on the PE (contraction 64 each), and a
            # single exp + single bf16 multiply cover both heads.  ebT =
            # exp(bias)^T comes bf16 from the host and folds multiplicatively.
            for qq in range(4):
                qs = slice(qq * 512, (qq + 1) * 512)
                bb = bigp.tile([P, H_PER_CORE, KT, 512], bf16, tag="big",
                               name=f"bb{qq}")
                nc.sync.dma_start(
                    out=bb,
                    in_=biasT_d[:, :, qs].rearrange("h (kt p) q -> p h kt q", p=P),
                )
                wps = [pW.tile([P, 512], f32, tag="pw", name=f"wps{qq}_{_h}")
                       for _h in range(H_PER_CORE)]
                for kt in range(KT):
                    ks = slice(kt * P, (kt + 1) * P)
                    lpp = pL.tile([P, H_PER_CORE, 512], f32, tag="pl")
                    for h in range(H_PER_CORE):
                        hs = slice(h * KD, (h + 1) * KD)
                        nc.tensor.matmul(
                            lpp[:, h, :],
                            lhsT=kT2[hs, ks], rhs=qT2[hs, qs],
                            start=True, stop=True,
                        )
                    etr = etp.tile([P, H_PER_CORE, 512], bf16, tag="etr")
                    nc.scalar.activation(
                        out=etr, in_=lpp, func=mybir.ActivationFunctionType.Exp,
                    )
                    et = etp.tile([P, H_PER_CORE, 512], bf16, tag="et")
                    mul_eng = nc.vector if kt % 2 == 0 else nc.gpsimd
                    mul_eng.tensor_mul(out=et, in0=etr, in1=bb[:, :, kt, :])
                    for h in range(H_PER_CORE):
                        nc.tensor.matmul(
                            wps[h][0:VD + 1, :],
                            lhsT=v_sb[:, h, kt, :],
                            rhs=et[:, h, :],
                            start=(kt == 0), stop=(kt == KT - 1),
                        )
                # fixup: transpose [v.w | sums]^T back to [q, c], divide, gate
                for h in range(H_PER_CORE):
                    hs = slice(h * KD, (h + 1) * KD)
                    wsb = wsbp.tile([P, 512], f32, tag="wsb")
                    nc.vector.tensor_copy(out=wsb[0:VD + 1, :], in_=wps[h][0:VD + 1, :])
                    tp4 = pL.tile([P, 4, VD + 1], f32, tag="pl", name=f"tp4_{qq}_{h}")
                    for qb in range(4):
                        nc.tensor.matmul(
                            tp4[:, qb, :],
                            lhsT=wsb[0:VD + 1, qb * P:(qb + 1) * P],
                            rhs=id_f32[0:VD + 1, 0:VD + 1],
                            is_transpose=True, start=True, stop=True,
                        )
                    rec4 = smallp.tile([P, 4], f32, tag="rec")
                    nc.vector.reciprocal(out=rec4, in_=tp4[:, :, VD])
                    tmp4 = smallp.tile([P, 4, VD], f32, tag="tmp")
                    for qb in range(4):
                        nc.vector.tensor_scalar_mul(
                            out=tmp4[:, qb, :], in0=tp4[:, qb, 0:VD],
                            scalar1=rec4[:, qb:qb + 1])
                    nc.vector.tensor_mul(
                        out=out_sb[:, qq * 4:(qq + 1) * 4, hs],
                        in0=tmp4,
                        in1=gate_sb[:, qq * 4:(qq + 1) * 4, hs],
                    )

            nc.sync.dma_start(
                out=o_d[:, :].rearrange("(qt p) m -> p qt m", p=P), in_=out_sb,
            )
    return nc


_NC = None


def _get_nc():
    global _NC
    if _NC is None:
        _NC = build_nc()
    return _NC


# --- host side -------------------------------------------------------------
def prepare_in_maps(q_data, m_data, batched_bias, query_w, query_b, key_w,
                    value_w, gating_w):
    q = np.asarray(q_data, np.float32)[0]          # [NQ, A]
    m = np.asarray(m_data, np.float32)[0]          # [NK, A]
    bias = np.asarray(batched_bias, np.float32)[0]  # [H, NQ, NK]
    bq = np.asarray(query_b, np.float32)[0]        # [H, KD]

    qT = np.ascontiguousarray(q.T).astype(BF16)
    mT = np.ascontiguousarray(m.T).astype(BF16)

    def wslice(w, c):
        w = np.asarray(w, np.float32)
        return np.ascontiguousarray(
            w[:, 2 * c:2 * c + 2, :].reshape(A_DIM, HC)).astype(BF16)

    in_maps = []
    for c in range(NCORES):
        bT = np.ascontiguousarray(
            np.exp(bias[2 * c:2 * c + 2].transpose(0, 2, 1))).astype(BF16)
        in_maps.append({
            "qT": qT,
            "mT": mT,
            "biasT": bT,
            "wq": wslice(query_w, c),
            "wk": wslice(key_w, c),
            "wv": wslice(value_w, c),
            "wg": wslice(gating_w, c),
            "bq": np.ascontiguousarray(bq[2 * c:2 * c + 2].reshape(HC, 1)),
        })
    return in_maps


def gather_out(results):
    parts = [np.asarray(r["o"]).reshape(NQ, H_PER_CORE, VD) for r in results]
    return np.concatenate(parts, axis=1)[None].astype(np.float32)


def kernel(**inputs):
    in_maps = prepare_in_maps(**inputs)
    res = run_bass_kernel_spmd(_get_nc(), in_maps, core_ids=list(range(NCORES)))
    return gather_out(res.results)


# revision 15
# speedup vs baseline: 1.0940x; 1.0940x over previous
"""Trainium2 Bass kernel for BatchedCrossAttentionXSMM.

Reference computation (B=1, NQ=NK=2048, A=M=1024, H=16, KD=VD=64):
    q = (q_data @ query_w + query_b) * kd^-0.5      [Q, H, KD]
    k = m_data @ key_w                               [K, H, KD]
    v = m_data @ value_w                             [K, H, VD]
    logits = q k^T + bias                            [H, Q, K]
    w = softmax(logits, axis=-1)
    out = sigmoid(q_data @ gating_w) * (w @ v)       [Q, H, VD]

Sharding: tensor-parallel over heads -- 2 heads per NeuronCore, 8 cores.
Each core gets the full activations (pre-transposed + bf16 on host) and its
2 heads' weight/bias slices; outputs are concatenated on the host.

On-device layout trick: logits are computed *transposed* ([k, q]) as
K Q^T, accumulated on top of PSUM pre-seeded with bias^T (host supplies
bias already transposed; seeding is an identity matmul so `has_written`
semantics allow the K Q^T accumulation).  exp() then lands E^T in SBUF
directly in the layout the PV matmul needs; a [v | 1] stationary operand
produces both weighted values and softmax denominators in one PE pass.
"""

import re
import sys

for _p in ("/opt/trn_rl_repo",):
    if _p not in sys.path:
        sys.path.insert(0, _p)

import ml_dtypes
import numpy as np

import concourse.bass as bass
import concourse.mybir as mybir
import concourse.tile as tile
from concourse.bass_utils import run_bass_kernel_spmd
from concourse.masks import make_identity

BF16 = ml_dtypes.bfloat16
dt = mybir.dt

NCORES = 8
H_PER_CORE = 2
NQ = NK = 2048
A_DIM = 1024
KD = VD = 64
HC = H_PER_CORE * KD  # 128
SCALE = float(KD) ** -0.5
P = 128
AT = A_DIM // P  # 8 a-subtiles
QT = NQ // P     # 16 token tiles
KT = NK // P


# --- Tile tail-drain patch -------------------------------------------------
# The walrus build in this image caps sem-waits per instruction at 2; Tile's
# kernel-tail drain attaches one wait per live semaphore to a single Drain,
# which fails codegen ("Too many sync wait commands").  Spread the waits over
# a chain of SP nops (1 wait each) before the drain instead.
def _patched_drain_and_barrier(self, tick_clock, wait_clock):
    nc = self.nc
    gc = tick_clock.global_clock
    vals = [int(v) for v in re.findall(r"\d+", repr(gc))]
    alloc = self.sems.allocated()
    waits = []
    for proc, sem in alloc.items():
        v = vals[proc] if proc < len(vals) else 0
        if v > 0:
            mult = 16 if "DMA" in sem.name else 1
            waits.append((sem, v * mult))
    for sem, val in waits:
        nc.sync.nop(nofuse=True).wait_op(sem, val, "sem-ge")
    nc.sync.drain()
    nc.all_engine_barrier()
    popped = nc._tile_sem_poison_stack.pop()
    assert popped is self._sem_poison
    nc.clear_and_free_semaphores(list(self.sems.allocated().values()))
    nc.all_engine_barrier()


tile.TileContext._drain_and_barrier = _patched_drain_and_barrier


# --- BIR wait-splitting pass ----------------------------------------------
# Tile's wait assignment can attach 3+ semaphore waits to a single
# instruction; this walrus build encodes at most 2 wait commands per
# instruction.  Rewrite the serialized BIR: hoist excess waits onto
# preceding EventSemaphore instructions on the same engine.
_MAXW = 1
_orig_to_json_bytes = bass.Bass.to_json_bytes


def _to_json_bytes_split_waits(self):
    import json

    data = json.loads(_orig_to_json_bytes(self))
    ctr = 0
    for fn in data.get("functions", []):
        for bb in fn.get("blocks", []):
            newl = []
            for ins in bb["instructions"]:
                si = ins.get("sync_info")
                if si and si.get("on_wait") and len(si["on_wait"]) > _MAXW:
                    waits = si["on_wait"]
                    extra, keep = waits[:-_MAXW], waits[-_MAXW:]
                    for i in range(0, len(extra), _MAXW):
                        ctr += 1
                        newl.append({
                            "debug": ins.get("debug", 0),
                            "engine": ins["engine"],
                            "ins": [],
                            "outs": [],
                            "name": f"{ins['name']}-wsplit{ctr}",
                            "opcode": "EventSemaphore",
                            "sync_info": {
                                "on_update": [],
                                "on_wait": extra[i:i + _MAXW],
                            },
                        })
                    si["on_wait"] = keep
                newl.append(ins)
            bb["instructions"] = newl
    return json.dumps(data).encode()


bass.Bass.to_json_bytes = _to_json_bytes_split_waits


# --- device program --------------------------------------------------------
def build_nc():
    nc = bass.Bass()
    f32, bf16 = dt.float32, dt.bfloat16

    qT_d = nc.dram_tensor("qT", [A_DIM, NQ], bf16, kind="ExternalInput")
    mT_d = nc.dram_tensor("mT", [A_DIM, NK], bf16, kind="ExternalInput")
    biasT_d = nc.dram_tensor("biasT", [H_PER_CORE, NK, NQ], bf16, kind="ExternalInput")
    wq_d = nc.dram_tensor("wq", [A_DIM, HC], bf16, kind="ExternalInput")
    wk_d = nc.dram_tensor("wk", [A_DIM, HC], bf16, kind="ExternalInput")
    wv_d = nc.dram_tensor("wv", [A_DIM, HC], bf16, kind="ExternalInput")
    wg_d = nc.dram_tensor("wg", [A_DIM, HC], bf16, kind="ExternalInput")
    bq_d = nc.dram_tensor("bq", [HC, 1], f32, kind="ExternalInput")
    o_d = nc.dram_tensor("o", [NQ, HC], f32, kind="ExternalOutput")

    with tile.TileContext(nc) as tc:
        with (
            tc.tile_pool(name="consts", bufs=1) as consts,
            tc.tile_pool(name="bigp", bufs=3) as bigp,
            tc.tile_pool(name="etp", bufs=4) as etp,
            tc.tile_pool(name="wsbp", bufs=2) as wsbp,
            tc.tile_pool(name="smallp", bufs=4) as smallp,
            tc.tile_pool(name="pL", bufs=3, space="PSUM") as pL,
            tc.tile_pool(name="pW", bufs=2, space="PSUM") as pW,
        ):
            # ---- constants / persistent SBUF ----
            qT_sb = bigp.tile([P, AT, NQ], bf16, tag="big")
            mT_sb = bigp.tile([P, AT, NK], bf16, tag="big")
            nc.sync.dma_start(out=qT_sb, in_=qT_d[:, :].rearrange("(at p) n -> p at n", p=P))
            nc.sync.dma_start(out=mT_sb, in_=mT_d[:, :].rearrange("(at p) n -> p at n", p=P))

            w_sb = {}
            for name, d in (("wq", wq_d), ("wk", wk_d), ("wv", wv_d), ("wg", wg_d)):
                t = consts.tile([P, AT, HC], bf16, tag=f"{name}_sb")
                nc.sync.dma_start(out=t, in_=d[:, :].rearrange("(at p) m -> p at m", p=P))
                w_sb[name] = t
            bq_sb = consts.tile([HC, 1], f32, tag="bq_sb")
            nc.sync.dma_start(out=bq_sb, in_=bq_d[:, :])

            id_bf = consts.tile([P, P], bf16, tag="id_bf")
            make_identity(nc, id_bf)
            id_f32 = consts.tile([P, P], f32, tag="id_f32")
            make_identity(nc, id_f32)

            qT2 = consts.tile([HC, NQ], bf16, tag="qT2")
            kT2 = consts.tile([HC, NK], bf16, tag="kT2")
            v_sb = consts.tile([P, H_PER_CORE, KT, VD + 1], bf16, tag="v_sb")
            gate_sb = consts.tile([P, QT, HC], f32, tag="gate_sb")
            out_sb = consts.tile([P, QT, HC], f32, tag="out_sb")

            vT2 = consts.tile([HC, NK], bf16, tag="vT2")
            gT2 = consts.tile([HC, NQ], bf16, tag="gT2")

            # ---- phase 1: projections ----
            # All four projections in [head*c, token] layout: stationary is the
            # weight slice (8 LDWs each), the pre-transposed activations stream.
            for ch in range(2):
                sl = slice(ch * 1024, (ch + 1) * 1024)
                ps_q = pL.tile([P, 1024], f32, tag="pl")
                ps_k = pL.tile([P, 1024], f32, tag="pl")
                for at in range(AT):
                    for hf in range(2):
                        fs = slice(hf * 512, (hf + 1) * 512)
                        gfs = slice(ch * 1024 + hf * 512, ch * 1024 + (hf + 1) * 512)
                        st, sp = (at == 0), (at == AT - 1)
                        nc.tensor.matmul(ps_q[:, fs], lhsT=w_sb["wq"][:, at, :],
                                         rhs=qT_sb[:, at, gfs], start=st, stop=sp)
                        nc.tensor.matmul(ps_k[:, fs], lhsT=w_sb["wk"][:, at, :],
                                         rhs=mT_sb[:, at, gfs], start=st, stop=sp)
                # (q + bq) * scale, cast to bf16
                nc.vector.tensor_scalar(
                    out=qT2[:, sl], in0=ps_q, scalar1=bq_sb, scalar2=SCALE,
                    op0=mybir.AluOpType.add, op1=mybir.AluOpType.mult,
                )
                nc.vector.tensor_copy(out=kT2[:, sl], in_=ps_k)
            for ch in range(2):
                sl = slice(ch * 1024, (ch + 1) * 1024)
                ps_v = pL.tile([P, 1024], f32, tag="pl")
                ps_g = pL.tile([P, 1024], f32, tag="pl")
                for at in range(AT):
                    for hf in range(2):
                        fs = slice(hf * 512, (hf + 1) * 512)
                        gfs = slice(ch * 1024 + hf * 512, ch * 1024 + (hf + 1) * 512)
                        st, sp = (at == 0), (at == AT - 1)
                        nc.tensor.matmul(ps_v[:, fs], lhsT=w_sb["wv"][:, at, :],
                                         rhs=mT_sb[:, at, gfs], start=st, stop=sp)
                        nc.tensor.matmul(ps_g[:, fs], lhsT=w_sb["wg"][:, at, :],
                                         rhs=qT_sb[:, at, gfs], start=st, stop=sp)
                nc.vector.tensor_copy(out=vT2[:, sl], in_=ps_v)
                # sigmoid(x) = 0.5 + 0.5*tanh(x/2): tanh shares the exp ACT
                # table set, so no table reload between phases.
                nc.scalar.activation(out=gT2[:, sl], in_=ps_g,
                                     func=mybir.ActivationFunctionType.Tanh,
                                     scale=0.5)
                nc.vector.tensor_scalar(
                    out=gT2[:, sl], in0=gT2[:, sl], scalar1=0.5, scalar2=0.5,
                    op0=mybir.AluOpType.mult, op1=mybir.AluOpType.add,
                )

            # transpose v^T back to natural [k, c] tiles with a ones column
            # (both heads in one full-128 transpose per k-tile).
            for kt in range(KT):
                tps = pL.tile([P, P], f32, tag="pl", name=f"vtp{kt}")
                nc.tensor.matmul(
                    tps, lhsT=vT2[:, kt * P:(kt + 1) * P],
                    rhs=id_bf, start=True, stop=True,
                )
                nc.vector.tensor_copy(
                    out=v_sb[:, :, kt, 0:VD],
                    in_=tps.rearrange("p (h c) -> p h c", h=H_PER_CORE),
                )
            nc.vector.memset(v_sb[:, :, :, VD:VD + 1], 1.0)

            # transpose gate^T back to natural [q, head*c] (both heads at once).
            for qt in range(QT):
                tps = pL.tile([P, P], f32, tag="pl")
                nc.tensor.matmul(
                    tps, lhsT=gT2[:, qt * P:(qt + 1) * P], rhs=id_bf,
                    start=True, stop=True,
                )
                nc.vector.tensor_copy(out=gate_sb[:, qt, :], in_=tps)

            # ---- phase 2: attention (q in quarters, heads paired) ----
            # Both heads' logits live in one [P, 2, 512] PSUM tile: the KQ^T
            # pair is row-group packed on the PE (contraction 64 each), and a
            # single exp + single bf16 multiply cover both heads.  ebT =
            # exp(bias)^T comes bf16 from the host and folds multiplicatively.
            for qq in range(4):
                qs = slice(qq * 512, (qq + 1) * 512)
                bb = bigp.tile([P, H_PER_CORE, KT, 512], bf16, tag="big",
                               name=f"bb{qq}")
                nc.sync.dma_start(
                    out=bb,
                    in_=biasT_d[:, :, qs].rearrange("h (kt p) q -> p h kt q", p=P),
                )
                wps = [pW.tile([P, 512], f32, tag="pw", name=f"wps{qq}_{_h}")
                       for _h in range(H_PER_CORE)]
                for kt in range(KT):
                    ks = slice(kt * P, (kt + 1) * P)
                    lpp = pL.tile([P, H_PER_CORE, 512], f32, tag="pl")
                    for h in range(H_PER_CORE):
                        hs = slice(h * KD, (h + 1) * KD)
                        nc.tensor.matmul(
                            lpp[:, h, :],
                            lhsT=kT2[hs, ks], rhs=qT2[hs, qs],
                            start=True, stop=True,
                        )
                    etr = etp.tile([P, H_PER_CORE, 512], bf16, tag="etr")
                    nc.scalar.activation(
                        out=etr, in_=lpp, func=mybir.ActivationFunctionType.Exp,
                    )
                    et = etp.tile([P, H_PER_CORE, 512], bf16, tag="et")
                    nc.vector.tensor_mul(out=et, in0=etr, in1=bb[:, :, kt, :])
                    for h in range(H_PER_CORE):
                        nc.tensor.matmul(
                            wps[h][0:VD + 1, :],
                            lhsT=v_sb[:, h, kt, :],
                            rhs=et[:, h, :],
                            start=(kt == 0), stop=(kt == KT - 1),
                        )
                # fixup: transpose [v.w | sums]^T back to [q, c], divide, gate
                for h in range(H_PER_CORE):
                    hs = slice(h * KD, (h + 1) * KD)
                    wsb = wsbp.tile([P, 512], f32, tag="wsb")
                    nc.vector.tensor_copy(out=wsb[0:VD + 1, :], in_=wps[h][0:VD + 1, :])
                    tp4 = pL.tile([P, 4, VD + 1], f32, tag="pl", name=f"tp4_{qq}_{h}")
                    for qb in range(4):
                        nc.tensor.matmul(
                            tp4[:, qb, :],
                            lhsT=wsb[0:VD + 1, qb * P:(qb + 1) * P],
                            rhs=id_f32[0:VD + 1, 0:VD + 1],
                            is_transpose=True, start=True, stop=True,
                        )
                    rec4 = smallp.tile([P, 4], f32, tag="rec")
                    nc.vector.reciprocal(out=rec4, in_=tp4[:, :, VD])
                    tmp4 = smallp.tile([P, 4, VD], f32, tag="tmp")
                    for qb in range(4):
                        nc.vector.tensor_scalar_mul(
                            out=tmp4[:, qb, :], in0=tp4[:, qb, 0:VD],
                            scalar1=rec4[:, qb:qb + 1])
                    nc.vector.tensor_mul(
                        out=out_sb[:, qq * 4:(qq + 1) * 4, hs],
                        in0=tmp4,
                        in1=gate_sb[:, qq * 4:(qq + 1) * 4, hs],
                    )

            nc.sync.dma_start(
                out=o_d[:, :].rearrange("(qt p) m -> p qt m", p=P), in_=out_sb,
            )
    return nc


_NC = None


def _get_nc():
    global _NC
    if _NC is None:
        _NC = build_nc()
    return _NC


# --- host side -------------------------------------------------------------
def prepare_in_maps(q_data, m_data, batched_bias, query_w, query_b, key_w,
                    value_w, gating_w):
    q = np.asarray(q_data, np.float32)[0]          # [NQ, A]
    m = np.asarray(m_data, np.float32)[0]          # [NK, A]
    bias = np.asarray(batched_bias, np.float32)[0]  # [H, NQ, NK]
    bq = np.asarray(query_b, np.float32)[0]        # [H, KD]

    qT = np.ascontiguousarray(q.T).astype(BF16)
    mT = np.ascontiguousarray(m.T).astype(BF16)

    def wslice(w, c):
        w = np.asarray(w, np.float32)
        return np.ascontiguousarray(
            w[:, 2 * c:2 * c + 2, :].reshape(A_DIM, HC)).astype(BF16)

    in_maps = []
    for c in range(NCORES):
        bT = np.ascontiguousarray(
            np.exp(bias[2 * c:2 * c + 2].transpose(0, 2, 1))).astype(BF16)
        in_maps.append({
            "qT": qT,
            "mT": mT,
            "biasT": bT,
            "wq": wslice(query_w, c),
            "wk": wslice(key_w, c),
            "wv": wslice(value_w, c),
            "wg": wslice(gating_w, c),
            "bq": np.ascontiguousarray(bq[2 * c:2 * c + 2].reshape(HC, 1)),
        })
    return in_maps


def gather_out(results):
    parts = [np.asarray(r["o"]).reshape(NQ, H_PER_CORE, VD) for r in results]
    return np.concatenate(parts, axis=1)[None].astype(np.float32)


def kernel(**inputs):
    in_maps = prepare_in_maps(**inputs)
    res = run_bass_kernel_spmd(_get_nc(), in_maps, core_ids=list(range(NCORES)))
    return gather_out(res.results)


# revision 16
# speedup vs baseline: 1.1264x; 1.0296x over previous
"""Trainium2 Bass kernel for BatchedCrossAttentionXSMM.

Reference computation (B=1, NQ=NK=2048, A=M=1024, H=16, KD=VD=64):
    q = (q_data @ query_w + query_b) * kd^-0.5      [Q, H, KD]
    k = m_data @ key_w                               [K, H, KD]
    v = m_data @ value_w                             [K, H, VD]
    logits = q k^T + bias                            [H, Q, K]
    w = softmax(logits, axis=-1)
    out = sigmoid(q_data @ gating_w) * (w @ v)       [Q, H, VD]

Sharding: tensor-parallel over heads -- 2 heads per NeuronCore, 8 cores.
Each core gets the full activations (pre-transposed + bf16 on host) and its
2 heads' weight/bias slices; outputs are concatenated on the host.

On-device layout trick: logits are computed *transposed* ([k, q]) as
K Q^T, accumulated on top of PSUM pre-seeded with bias^T (host supplies
bias already transposed; seeding is an identity matmul so `has_written`
semantics allow the K Q^T accumulation).  exp() then lands E^T in SBUF
directly in the layout the PV matmul needs; a [v | 1] stationary operand
produces both weighted values and softmax denominators in one PE pass.
"""

import re
import sys

for _p in ("/opt/trn_rl_repo",):
    if _p not in sys.path:
        sys.path.insert(0, _p)

import ml_dtypes
import numpy as np

import concourse.bass as bass
import concourse.mybir as mybir
import concourse.tile as tile
from concourse.bass_utils import run_bass_kernel_spmd
from concourse.masks import make_identity

BF16 = ml_dtypes.bfloat16
dt = mybir.dt

NCORES = 8
H_PER_CORE = 2
NQ = NK = 2048
A_DIM = 1024
KD = VD = 64
HC = H_PER_CORE * KD  # 128
SCALE = float(KD) ** -0.5
P = 128
AT = A_DIM // P  # 8 a-subtiles
QT = NQ // P     # 16 token tiles
KT = NK // P


# --- Tile tail-drain patch -------------------------------------------------
# The walrus build in this image caps sem-waits per instruction at 2; Tile's
# kernel-tail drain attaches one wait per live semaphore to a single Drain,
# which fails codegen ("Too many sync wait commands").  Spread the waits over
# a chain of SP nops (1 wait each) before the drain instead.
def _patched_drain_and_barrier(self, tick_clock, wait_clock):
    nc = self.nc
    gc = tick_clock.global_clock
    vals = [int(v) for v in re.findall(r"\d+", repr(gc))]
    alloc = self.sems.allocated()
    waits = []
    for proc, sem in alloc.items():
        v = vals[proc] if proc < len(vals) else 0
        if v > 0:
            mult = 16 if "DMA" in sem.name else 1
            waits.append((sem, v * mult))
    for sem, val in waits:
        nc.sync.nop(nofuse=True).wait_op(sem, val, "sem-ge")
    nc.sync.drain()
    nc.all_engine_barrier()
    popped = nc._tile_sem_poison_stack.pop()
    assert popped is self._sem_poison
    nc.clear_and_free_semaphores(list(self.sems.allocated().values()))
    nc.all_engine_barrier()


tile.TileContext._drain_and_barrier = _patched_drain_and_barrier


# --- BIR wait-splitting pass ----------------------------------------------
# Tile's wait assignment can attach 3+ semaphore waits to a single
# instruction; this walrus build encodes at most 2 wait commands per
# instruction.  Rewrite the serialized BIR: hoist excess waits onto
# preceding EventSemaphore instructions on the same engine.
_MAXW = 1
_orig_to_json_bytes = bass.Bass.to_json_bytes


def _to_json_bytes_split_waits(self):
    import json

    data = json.loads(_orig_to_json_bytes(self))
    ctr = 0
    for fn in data.get("functions", []):
        for bb in fn.get("blocks", []):
            newl = []
            for ins in bb["instructions"]:
                si = ins.get("sync_info")
                if si and si.get("on_wait") and len(si["on_wait"]) > _MAXW:
                    waits = si["on_wait"]
                    extra, keep = waits[:-_MAXW], waits[-_MAXW:]
                    for i in range(0, len(extra), _MAXW):
                        ctr += 1
                        newl.append({
                            "debug": ins.get("debug", 0),
                            "engine": ins["engine"],
                            "ins": [],
                            "outs": [],
                            "name": f"{ins['name']}-wsplit{ctr}",
                            "opcode": "EventSemaphore",
                            "sync_info": {
                                "on_update": [],
                                "on_wait": extra[i:i + _MAXW],
                            },
                        })
                    si["on_wait"] = keep
                newl.append(ins)
            bb["instructions"] = newl
    return json.dumps(data).encode()


bass.Bass.to_json_bytes = _to_json_bytes_split_waits


# --- device program --------------------------------------------------------
def build_nc():
    nc = bass.Bass()
    f32, bf16 = dt.float32, dt.bfloat16

    qT_d = nc.dram_tensor("qT", [A_DIM, NQ], bf16, kind="ExternalInput")
    mT_d = nc.dram_tensor("mT", [A_DIM, NK], bf16, kind="ExternalInput")
    biasT_d = nc.dram_tensor("biasT", [H_PER_CORE, NK, NQ], bf16, kind="ExternalInput")
    wq_d = nc.dram_tensor("wq", [A_DIM, HC], bf16, kind="ExternalInput")
    wk_d = nc.dram_tensor("wk", [A_DIM, HC], bf16, kind="ExternalInput")
    wv_d = nc.dram_tensor("wv", [A_DIM, HC], bf16, kind="ExternalInput")
    wg_d = nc.dram_tensor("wg", [A_DIM, HC], bf16, kind="ExternalInput")
    bq_d = nc.dram_tensor("bq", [HC, 1], f32, kind="ExternalInput")
    o_d = nc.dram_tensor("o", [NQ, HC], f32, kind="ExternalOutput")

    with tile.TileContext(nc) as tc:
        with (
            tc.tile_pool(name="consts", bufs=1) as consts,
            tc.tile_pool(name="bigp", bufs=3) as bigp,
            tc.tile_pool(name="etp", bufs=4) as etp,
            tc.tile_pool(name="wsbp", bufs=2) as wsbp,
            tc.tile_pool(name="smallp", bufs=4) as smallp,
            tc.tile_pool(name="pL", bufs=3, space="PSUM") as pL,
            tc.tile_pool(name="pW", bufs=2, space="PSUM") as pW,
        ):
            # ---- constants / persistent SBUF ----
            qT_sb = bigp.tile([P, AT, NQ], bf16, tag="big")
            mT_sb = bigp.tile([P, AT, NK], bf16, tag="big")
            nc.sync.dma_start(out=qT_sb, in_=qT_d[:, :].rearrange("(at p) n -> p at n", p=P))
            nc.sync.dma_start(out=mT_sb, in_=mT_d[:, :].rearrange("(at p) n -> p at n", p=P))

            w_sb = {}
            for name, d in (("wq", wq_d), ("wk", wk_d), ("wv", wv_d), ("wg", wg_d)):
                t = consts.tile([P, AT, HC], bf16, tag=f"{name}_sb")
                nc.sync.dma_start(out=t, in_=d[:, :].rearrange("(at p) m -> p at m", p=P))
                w_sb[name] = t
            bq_sb = consts.tile([HC, 1], f32, tag="bq_sb")
            nc.sync.dma_start(out=bq_sb, in_=bq_d[:, :])

            id_bf = consts.tile([P, P], bf16, tag="id_bf")
            make_identity(nc, id_bf)
            id_f32 = consts.tile([P, P], f32, tag="id_f32")
            make_identity(nc, id_f32)

            qT2 = consts.tile([HC, NQ], bf16, tag="qT2")
            kT2 = consts.tile([HC, NK], bf16, tag="kT2")
            v_sb = consts.tile([P, H_PER_CORE, KT, VD + 1], bf16, tag="v_sb")
            gate_sb = consts.tile([P, QT, HC], f32, tag="gate_sb")
            out_sb = consts.tile([P, QT, HC], f32, tag="out_sb")

            vT2 = consts.tile([HC, NK], bf16, tag="vT2")
            gT2 = consts.tile([HC, NQ], bf16, tag="gT2")

            # ---- phase 1: projections ----
            # All four projections in [head*c, token] layout: stationary is the
            # weight slice (8 LDWs each), the pre-transposed activations stream.
            for ch in range(2):
                sl = slice(ch * 1024, (ch + 1) * 1024)
                ps_q = pL.tile([P, 1024], f32, tag="pl")
                ps_k = pL.tile([P, 1024], f32, tag="pl")
                for at in range(AT):
                    for hf in range(2):
                        fs = slice(hf * 512, (hf + 1) * 512)
                        gfs = slice(ch * 1024 + hf * 512, ch * 1024 + (hf + 1) * 512)
                        st, sp = (at == 0), (at == AT - 1)
                        nc.tensor.matmul(ps_q[:, fs], lhsT=w_sb["wq"][:, at, :],
                                         rhs=qT_sb[:, at, gfs], start=st, stop=sp)
                        nc.tensor.matmul(ps_k[:, fs], lhsT=w_sb["wk"][:, at, :],
                                         rhs=mT_sb[:, at, gfs], start=st, stop=sp)
                # (q + bq) * scale, cast to bf16
                nc.vector.tensor_scalar(
                    out=qT2[:, sl], in0=ps_q, scalar1=bq_sb, scalar2=SCALE,
                    op0=mybir.AluOpType.add, op1=mybir.AluOpType.mult,
                )
                nc.vector.tensor_copy(out=kT2[:, sl], in_=ps_k)
            for ch in range(4):
                sl = slice(ch * 512, (ch + 1) * 512)
                ps_v = pW.tile([P, 512], f32, tag="pw")
                ps_g = pW.tile([P, 512], f32, tag="pw")
                for at in range(AT):
                    st, sp = (at == 0), (at == AT - 1)
                    nc.tensor.matmul(ps_v, lhsT=w_sb["wv"][:, at, :],
                                     rhs=mT_sb[:, at, sl], start=st, stop=sp)
                    nc.tensor.matmul(ps_g, lhsT=w_sb["wg"][:, at, :],
                                     rhs=qT_sb[:, at, sl], start=st, stop=sp)
                nc.vector.tensor_copy(out=vT2[:, sl], in_=ps_v)
                # sigmoid(x) = 0.5 + 0.5*tanh(x/2): tanh shares the exp ACT
                # table set, so no table reload between phases.
                nc.scalar.activation(out=gT2[:, sl], in_=ps_g,
                                     func=mybir.ActivationFunctionType.Tanh,
                                     scale=0.5)
                nc.vector.tensor_scalar(
                    out=gT2[:, sl], in0=gT2[:, sl], scalar1=0.5, scalar2=0.5,
                    op0=mybir.AluOpType.mult, op1=mybir.AluOpType.add,
                )

            # transpose v^T back to natural [k, c] tiles with a ones column
            # (both heads in one full-128 transpose per k-tile).
            for kt in range(KT):
                tps = pL.tile([P, P], f32, tag="pl", name=f"vtp{kt}")
                nc.tensor.matmul(
                    tps, lhsT=vT2[:, kt * P:(kt + 1) * P],
                    rhs=id_bf, start=True, stop=True,
                )
                nc.vector.tensor_copy(
                    out=v_sb[:, :, kt, 0:VD],
                    in_=tps.rearrange("p (h c) -> p h c", h=H_PER_CORE),
                )
            nc.vector.memset(v_sb[:, :, :, VD:VD + 1], 1.0)

            # transpose gate^T back to natural [q, head*c] (both heads at once).
            for qt in range(QT):
                tps = pL.tile([P, P], f32, tag="pl")
                nc.tensor.matmul(
                    tps, lhsT=gT2[:, qt * P:(qt + 1) * P], rhs=id_bf,
                    start=True, stop=True,
                )
                nc.vector.tensor_copy(out=gate_sb[:, qt, :], in_=tps)

            # ---- phase 2: attention (q in quarters, heads paired) ----
            # Both heads' logits live in one [P, 2, 512] PSUM tile: the KQ^T
            # pair is row-group packed on the PE (contraction 64 each), and a
            # single exp + single bf16 multiply cover both heads.  ebT =
            # exp(bias)^T comes bf16 from the host and folds multiplicatively.
            for qq in range(4):
                qs = slice(qq * 512, (qq + 1) * 512)
                bb = bigp.tile([P, H_PER_CORE, KT, 512], bf16, tag="big",
                               name=f"bb{qq}")
                nc.sync.dma_start(
                    out=bb,
                    in_=biasT_d[:, :, qs].rearrange("h (kt p) q -> p h kt q", p=P),
                )
                wps = [pW.tile([P, 512], f32, tag="pw", name=f"wps{qq}_{_h}")
                       for _h in range(H_PER_CORE)]
                for kt in range(KT):
                    ks = slice(kt * P, (kt + 1) * P)
                    lpp = pL.tile([P, H_PER_CORE, 512], f32, tag="pl")
                    for h in range(H_PER_CORE):
                        hs = slice(h * KD, (h + 1) * KD)
                        nc.tensor.matmul(
                            lpp[:, h, :],
                            lhsT=kT2[hs, ks], rhs=qT2[hs, qs],
                            start=True, stop=True,
                        )
                    etr = etp.tile([P, H_PER_CORE, 512], bf16, tag="etr")
                    nc.scalar.activation(
                        out=etr, in_=lpp, func=mybir.ActivationFunctionType.Exp,
                    )
                    et = etp.tile([P, H_PER_CORE, 512], bf16, tag="et")
                    nc.vector.tensor_mul(out=et, in0=etr, in1=bb[:, :, kt, :])
                    for h in range(H_PER_CORE):
                        nc.tensor.matmul(
                            wps[h][0:VD + 1, :],
                            lhsT=v_sb[:, h, kt, :],
                            rhs=et[:, h, :],
                            start=(kt == 0), stop=(kt == KT - 1),
                        )
                # fixup: transpose [v.w | sums]^T back to [q, c], divide, gate
                for h in range(H_PER_CORE):
                    hs = slice(h * KD, (h + 1) * KD)
                    wsb = wsbp.tile([P, 512], f32, tag="wsb")
                    nc.vector.tensor_copy(out=wsb[0:VD + 1, :], in_=wps[h][0:VD + 1, :])
                    tp4 = pL.tile([P, 4, VD + 1], f32, tag="pl", name=f"tp4_{qq}_{h}")
                    for qb in range(4):
                        nc.tensor.matmul(
                            tp4[:, qb, :],
                            lhsT=wsb[0:VD + 1, qb * P:(qb + 1) * P],
                            rhs=id_f32[0:VD + 1, 0:VD + 1],
                            is_transpose=True, start=True, stop=True,
                        )
                    rec4 = smallp.tile([P, 4], f32, tag="rec")
                    nc.vector.reciprocal(out=rec4, in_=tp4[:, :, VD])
                    tmp4 = smallp.tile([P, 4, VD], f32, tag="tmp")
                    for qb in range(4):
                        nc.vector.tensor_scalar_mul(
                            out=tmp4[:, qb, :], in0=tp4[:, qb, 0:VD],
                            scalar1=rec4[:, qb:qb + 1])
                    nc.vector.tensor_mul(
                        out=out_sb[:, qq * 4:(qq + 1) * 4, hs],
                        in0=tmp4,
                        in1=gate_sb[:, qq * 4:(qq + 1) * 4, hs],
                    )
                out_q = o_d[:, :].rearrange("(qt p) m -> p qt m", p=P)
                nc.sync.dma_start(
                    out=out_q[:, qq * 4:(qq + 1) * 4, :],
                    in_=out_sb[:, qq * 4:(qq + 1) * 4, :],
                )


    return nc


_NC = None


def _get_nc():
    global _NC
    if _NC is None:
        _NC = build_nc()
    return _NC


# --- host side -------------------------------------------------------------
def prepare_in_maps(q_data, m_data, batched_bias, query_w, query_b, key_w,
                    value_w, gating_w):
    q = np.asarray(q_data, np.float32)[0]          # [NQ, A]
    m = np.asarray(m_data, np.float32)[0]          # [NK, A]
    bias = np.asarray(batched_bias, np.float32)[0]  # [H, NQ, NK]
    bq = np.asarray(query_b, np.float32)[0]        # [H, KD]

    qT = np.ascontiguousarray(q.T).astype(BF16)
    mT = np.ascontiguousarray(m.T).astype(BF16)

    def wslice(w, c):
        w = np.asarray(w, np.float32)
        return np.ascontiguousarray(
            w[:, 2 * c:2 * c + 2, :].reshape(A_DIM, HC)).astype(BF16)

    in_maps = []
    for c in range(NCORES):
        bT = np.ascontiguousarray(
            np.exp(bias[2 * c:2 * c + 2].transpose(0, 2, 1))).astype(BF16)
        in_maps.append({
            "qT": qT,
            "mT": mT,
            "biasT": bT,
            "wq": wslice(query_w, c),
            "wk": wslice(key_w, c),
            "wv": wslice(value_w, c),
            "wg": wslice(gating_w, c),
            "bq": np.ascontiguousarray(bq[2 * c:2 * c + 2].reshape(HC, 1)),
        })
    return in_maps


def gather_out(results):
    parts = [np.asarray(r["o"]).reshape(NQ, H_PER_CORE, VD) for r in results]
    return np.concatenate(parts, axis=1)[None].astype(np.float32)


def kernel(**inputs):
    in_maps = prepare_in_maps(**inputs)
    res = run_bass_kernel_spmd(_get_nc(), in_maps, core_ids=list(range(NCORES)))
    return gather_out(res.results)
